# revision 2
# baseline (speedup 1.0000x reference)
"""Causal multi-head attention (B=256, T=197, C=768, H=12, D=64) on 8 trn2 cores.

v2 strategy (scheme D — no DRAM-bounce normalization):
- Data-parallel over batch: 32 batches per core, no collectives.
- Host pre-transposes x to [C, T] layout per batch (xT); Q^T/K^T projections
  in [c, t] layout (4 batches at a time), V in [t, c] layout per batch.
- Per (batch, head), S^T in [k, q] layout with the causal mask folded into
  the SAME PSUM accumulation: a leading "mask matmul" (lhsT=I, rhs=Rcomb)
  deposits -30 on the masked triangles (start=True initializes the whole
  tile, so exp never sees garbage), then the two S matmuls accumulate.
- ONE exp per head over the whole [128, 266] tile (block0 k0x[q 0:197] at
  cols 0:197, block1 k1x[q 128:197] at cols 197:266).
- ctx computed in [q, d] layout: ctx[q, d+1] = P^T.T @ [V|1] with lhsT = the
  P^T tile itself. The ones-column of V makes col 64 the softmax denominator,
  which lands on the PARTITION axis (per-q) -> normalization is a cheap
  per-partition scale: DVE reciprocal + ACT/DVE copy-with-scale. No DMA
  round-trip, no partition broadcast.
- ctx^T for the output projection is rebuilt with 12 cheap regular matmuls
  against an identity (in_^T @ I), batched per batch (2 per c-block).
- out = ctx^T.T @ Wo, evict f32, DMA out. All matmuls bf16, PSUM f32.
"""

import numpy as np

B, T, C, H = 256, 197, 768, 12
D = C // H          # 64
P = 128             # partition size
CB = C // P         # 6 c-blocks
NCORES = 8
NB = B // NCORES    # 32 batches per core
G = 4               # batches per projection group
NG = NB // G        # 8 groups
TG = G * T          # 788 tokens per group
T0 = P              # first t/k block rows (128)
T1 = T - P          # second block rows (69)
SW = T + T1         # S tile width: 197 + 69 = 266
NEG = -30.0         # additive causal mask value

_CACHE = {}


def _split_ctrl_waits(nc):
    """This walrus encodes at most 1 sem wait per instruction (2 for
    EventSemaphore), but Tile emits instructions with several. Split excess
    waits onto NoOps inserted before the offending instruction on the same
    engine (a NoOp itself carries 1 wait)."""
    import concourse.mybir as mybir

    for fn in nc.m.functions:
        for bb in fn.blocks:
            insts = bb.instructions
            newlist = []
            changed = False
            for inst in insts:
                cap = 2 if isinstance(inst, mybir.InstEventSemaphore) else 1
                si = inst.sync_info
                waits = list(si.on_wait) if si and si.on_wait else []
                if len(waits) > cap:
                    changed = True
                    head, rest = waits[:-cap], waits[-cap:]
                    for w in head:
                        nop = mybir.InstNoOp(
                            name=nc.get_next_instruction_name(),
                            bass_nofuse=True,
                            engine=inst.engine,
                            sync_info=mybir.SyncInfo(on_wait=[w], on_update=[]),
                        )
                        newlist.append(nop)
                    inst.sync_info = mybir.SyncInfo(
                        on_wait=rest,
                        on_update=list(si.on_update) if si.on_update else [],
                    )
                newlist.append(inst)
            if changed:
                bb.instructions = newlist


def _dedup_ldweights(nc):
    """Delete an InstLdweights that reloads the exact weights already loaded
    by the previous PE ldweights with no different load in between. Only
    drops wait-free duplicates."""
    import concourse.mybir as mybir

    ndrop = 0
    for fn in nc.m.functions:
        for bb in fn.blocks:
            insts = bb.instructions
            newlist = []
            last_sig = None
            changed = False
            for inst in insts:
                if inst.engine != mybir.EngineType.PE:
                    newlist.append(inst)
                    continue
                if type(inst).__name__ == "InstLdweights":
                    si = inst.sync_info
                    nw = len(si.on_wait) if si and si.on_wait else 0
                    nu = len(si.on_update) if si and si.on_update else 0
                    sig = (str(inst.ins[0]), str(inst.tile_position),
                           str(inst.tile_size), str(inst.is_transpose),
                           str(inst.perf_mode))
                    if sig == last_sig and nw == 0 and nu == 0:
                        changed = True
                        ndrop += 1
                        continue  # drop duplicate
                    last_sig = sig
                newlist.append(inst)
            if changed:
                bb.instructions = newlist
    return ndrop


def build_nc(nb=NB, split_waits=True, repeat=1, parts=("qk", "v", "attn", "tr", "o")):
    import concourse.bass as bass
    import concourse.mybir as mybir
    from concourse.tile import TileContext

    f32 = mybir.dt.float32
    bf16 = mybir.dt.bfloat16
    Exp = mybir.ActivationFunctionType.Exp
    Copy = mybir.ActivationFunctionType.Copy

    ng = nb // G

    nc = bass.Bass()
    xT = nc.declare_dram_parameter("xT", [CB, P, nb * T], bf16, isOutput=False)
    wq = nc.declare_dram_parameter("wq", [CB, P, C], bf16, isOutput=False)
    wk = nc.declare_dram_parameter("wk", [CB, P, C], bf16, isOutput=False)
    wv = nc.declare_dram_parameter("wv", [CB, P, C], bf16, isOutput=False)
    wo = nc.declare_dram_parameter("wo", [CB, P, C], bf16, isOutput=False)
    maskp = nc.declare_dram_parameter("mask", [P, SW], bf16, isOutput=False)
    idp = nc.declare_dram_parameter("ident", [P, P], bf16, isOutput=False)
    out = nc.declare_dram_parameter("out", [nb, T, C], bf16, isOutput=True)

    with TileContext(nc) as tc:
        with (
            tc.tile_pool(name="weights", bufs=1) as wpool,
            tc.tile_pool(name="x", bufs=2) as xpool,
            tc.tile_pool(name="qk", bufs=2) as qkpool,
            tc.tile_pool(name="v", bufs=8) as vpool,
            tc.tile_pool(name="pT", bufs=4) as ppool,
            tc.tile_pool(name="inv", bufs=6) as invpool,
            tc.tile_pool(name="cq", bufs=2) as cqpool,
            tc.tile_pool(name="ctxt", bufs=2) as ctpool,
            tc.tile_pool(name="outsb", bufs=3) as outsbpool,
            tc.tile_pool(name="bigps", bufs=2, space="PSUM") as bigps,
            tc.tile_pool(name="sps", bufs=1, space="PSUM") as spool,
            tc.tile_pool(name="ctxps", bufs=1, space="PSUM") as ctxpool,
            tc.tile_pool(name="tps", bufs=1, space="PSUM") as tpool,
        ):
            # --- static tiles ---
            wq_sb = wpool.tile([P, CB, C], bf16, tag="wq")
            wk_sb = wpool.tile([P, CB, C], bf16, tag="wk")
            wv_sb = wpool.tile([P, CB, C], bf16, tag="wv")
            wo_sb = wpool.tile([P, CB, C], bf16, tag="wo")
            mask_sb = wpool.tile([P, SW], bf16, tag="mask")
            id_sb = wpool.tile([P, P], bf16, tag="ident")
            for dram, sb in ((wq, wq_sb), (wk, wk_sb), (wv, wv_sb), (wo, wo_sb)):
                nc.sync.dma_start(out=sb[:], in_=dram.rearrange("ib p c -> p ib c"))
            nc.sync.dma_start(out=mask_sb[:], in_=maskp[:])
            nc.sync.dma_start(out=id_sb[:], in_=idp[:])

            for g in range(ng * repeat):
                g = g % ng
                xT_sb = xpool.tile([P, CB, TG], bf16, tag="xT")
                nc.sync.dma_start(
                    out=xT_sb[:],
                    in_=xT[:, :, g * TG:(g + 1) * TG].rearrange("ib p t -> p ib t"),
                )

                # --- q^T / k^T projections, [c, t] layout, 4 batches at once ---
                qT_sb = qkpool.tile([P, CB, TG], bf16, tag="qT")
                kT_sb = qkpool.tile([P, CB, TG], bf16, tag="kT")
                if "qk" not in parts:
                    nc.vector.memset(qT_sb[:], 0.0)
                    nc.vector.memset(kT_sb[:], 0.0)
                for w_sb, dst in (((wq_sb, qT_sb), (wk_sb, kT_sb))
                                  if "qk" in parts else ()):
                    for cb in range(CB):
                        ps = bigps.tile([P, TG], f32, tag="ps")
                        for ib in range(CB):
                            lhs = w_sb[:, ib, cb * P:(cb + 1) * P]
                            nc.tensor.matmul(
                                ps[:, 0:512], lhsT=lhs, rhs=xT_sb[:, ib, 0:512],
                                start=(ib == 0), stop=(ib == CB - 1))
                            nc.tensor.matmul(
                                ps[:, 512:TG], lhsT=lhs,
                                rhs=xT_sb[:, ib, 512:TG],
                                start=(ib == 0), stop=(ib == CB - 1))
                        nc.scalar.activation(dst[:, cb, :], ps[:, :], Copy)

                # --- v projection, [t, c] layout, per batch ---
                vtiles = {}
                for b in range(G):
                    for tb in range(2):
                        rows = T0 if tb == 0 else T1
                        col0 = b * T + tb * P
                        if "v" in parts:
                            ps = bigps.tile([P, TG], f32, tag="ps")
                        else:
                            ps = None
                        for ib in (range(CB) if "v" in parts else ()):
                            lhs = xT_sb[:, ib, col0:col0 + rows]
                            nc.tensor.matmul(
                                ps[0:rows, 0:512], lhsT=lhs, rhs=wv_sb[:, ib, 0:512],
                                start=(ib == 0), stop=(ib == CB - 1))
                            nc.tensor.matmul(
                                ps[0:rows, 512:C], lhsT=lhs,
                                rhs=wv_sb[:, ib, 512:C],
                                start=(ib == 0), stop=(ib == CB - 1))
                        v_sb = vpool.tile([P, H, D + 1], bf16, tag="v")
                        if "v" in parts:
                            nc.scalar.activation(
                                v_sb[0:rows, :, 0:D],
                                ps[0:rows, 0:C].rearrange("p (h d) -> p h d", d=D), Copy)
                        else:
                            nc.vector.memset(v_sb[0:rows, :, 0:D], 0.01)
                        nc.vector.memset(v_sb[0:rows, :, D], 1.0)
                        vtiles[(b, tb)] = v_sb

                # --- attention per batch ---
                for b in range(G):
                    ctxq0 = cqpool.tile([P, C], bf16, tag="cq0")
                    ctxq1 = cqpool.tile([P, C], bf16, tag="cq1")
                    vb0 = vtiles[(b, 0)]
                    vb1 = vtiles[(b, 1)]
                    ctx3 = None
                    if "attn" not in parts:
                        nc.vector.memset(ctxq0[:], 0.01)
                        nc.vector.memset(ctxq1[:], 0.01)
                    for jp in (range(H // 2) if "attn" in parts else ()):
                        # s-pair tile: head i of the pair lives in PSUM bank
                        # i of this 2-bank tile, so ONE exp instruction can
                        # cover both heads with a 2D access pattern.
                        s = spool.tile([P, 2, 512], f32, tag="s")
                        # phase 1: mask + S matmuls for the head pair
                        for i in (0, 1):
                            # causal mask deposit; start=True defines the
                            # whole region (no garbage for the exp below)
                            nc.tensor.matmul(
                                s[:, i, 0:SW], lhsT=id_sb[:, :], rhs=mask_sb[:, :],
                                start=True, stop=False, skip_group_check=True)
                        for i in (0, 1):
                            base = i * D
                            j = jp
                            kh = kT_sb[base:base + D, j, b * T:(b + 1) * T]
                            qh = qT_sb[base:base + D, j, b * T:(b + 1) * T]
                            nc.tensor.matmul(
                                s[:, i, 0:T], lhsT=kh[:, 0:P], rhs=qh,
                                start=False, stop=False, skip_group_check=True)
                            nc.tensor.matmul(
                                s[0:T1, i, T:SW], lhsT=kh[:, P:T], rhs=qh[:, P:T],
                                start=False, stop=True, skip_group_check=True)
                        # phase 2: one exp instruction for the whole pair
                        pT = ppool.tile([P, 2, SW], bf16, tag="pT")
                        nc.scalar.activation(pT[:, :, :], s[:, :, 0:SW], Exp)
                        # phase 3: ctx in [q, d] layout + normalize
                        for i in (0, 1):
                            h = 2 * jp + i
                            slot = h % 3
                            if slot == 0:
                                ctx3 = ctxpool.tile([P, 390], f32, tag="ctx3")
                            cb0 = slot * 130
                            nc.tensor.matmul(
                                ctx3[:, cb0:cb0 + D + 1], lhsT=pT[:, i, 0:P],
                                rhs=vb0[:, h, :], start=True, stop=True,
                                skip_group_check=True)
                            nc.tensor.matmul(
                                ctx3[0:T1, cb0 + 65:cb0 + 65 + D + 1],
                                lhsT=pT[:, i, P:T], rhs=vb0[:, h, :],
                                start=True, stop=False, skip_group_check=True)
                            nc.tensor.matmul(
                                ctx3[0:T1, cb0 + 65:cb0 + 65 + D + 1],
                                lhsT=pT[0:T1, i, T:SW], rhs=vb1[0:T1, h, :],
                                start=False, stop=True, skip_group_check=True)
                            inv = invpool.tile([P, 2], mybir.dt.float32, tag="inv")
                            # one reciprocal covers both q-blocks' Z columns;
                            # rows 69:128 of the q1 column are garbage (may
                            # produce inf) and are never read.
                            nc.vector.reciprocal(
                                inv[:, 0:2],
                                ctx3[:, cb0 + D:cb0 + D + 66:65])
                            nc.scalar.activation(
                                ctxq0[:, h * D:(h + 1) * D],
                                ctx3[:, cb0:cb0 + D], Copy, scale=inv[:, 0:1])
                            nc.vector.tensor_scalar_mul(
                                ctxq1[0:T1, h * D:(h + 1) * D],
                                ctx3[0:T1, cb0 + 65:cb0 + 65 + D], inv[0:T1, 1:2])

                    # --- rebuild ctx^T via identity matmuls (2 c-blocks per
                    # PSUM tile so the eviction batches) ---
                    ctxT_sb = ctpool.tile([P, CB, T], bf16, tag="ctxT")
                    if "tr" not in parts:
                        nc.vector.memset(ctxT_sb[:], 0.01)
                    for cb2 in (range(CB // 2) if "tr" in parts else ()):
                        tt = tpool.tile([P, 2, T], f32, tag="tt")
                        for k in range(2):
                            cb = 2 * cb2 + k
                            nc.tensor.matmul(
                                tt[:, k, 0:P], lhsT=ctxq0[:, cb * P:(cb + 1) * P],
                                rhs=id_sb[:, :], start=True, stop=True,
                                skip_group_check=True)
                            nc.tensor.matmul(
                                tt[:, k, P:T], lhsT=ctxq1[0:T1, cb * P:(cb + 1) * P],
                                rhs=id_sb[0:T1, 0:T1], start=True, stop=True,
                                skip_group_check=True)
                        nc.vector.tensor_copy(
                            ctxT_sb[:, 2 * cb2:2 * cb2 + 2, :], tt[:, :, :])

                    # --- out = ctx^T.T @ Wo ---
                    for tb in (range(2) if "o" in parts else ()):
                        rows = T0 if tb == 0 else T1
                        ps = bigps.tile([P, TG], f32, tag="ps")
                        for j in range(CB):
                            lhs = ctxT_sb[:, j, tb * P:tb * P + rows]
                            nc.tensor.matmul(
                                ps[0:rows, 0:512], lhsT=lhs, rhs=wo_sb[:, j, 0:512],
                                start=(j == 0), stop=(j == CB - 1))
                            nc.tensor.matmul(
                                ps[0:rows, 512:C], lhsT=lhs,
                                rhs=wo_sb[:, j, 512:C],
                                start=(j == 0), stop=(j == CB - 1))
                        out_sb = outsbpool.tile([P, C], bf16, tag="out")
                        nc.vector.tensor_copy(out_sb[0:rows, :], ps[0:rows, 0:C])
                        nc.sync.dma_start(
                            out=out[g * G + b, tb * P:tb * P + rows, :],
                            in_=out_sb[0:rows, :])

    _dedup_ldweights(nc)
    if split_waits:
        _split_ctrl_waits(nc)
    return nc


def _prep_core_inputs(hidden_states, Wq, Wk, Wv, Wo):
    """Host-side layout prep. Returns per-core in_maps (list of dicts)."""
    import ml_dtypes

    bf16 = ml_dtypes.bfloat16
    scale = 1.0 / np.sqrt(D)
    x = np.ascontiguousarray(hidden_states.astype(np.float32))
    wq_h = np.ascontiguousarray((Wq * scale).reshape(CB, P, C).astype(bf16))
    wk_h = np.ascontiguousarray(Wk.reshape(CB, P, C).astype(bf16))
    wv_h = np.ascontiguousarray(Wv.reshape(CB, P, C).astype(bf16))
    wo_h = np.ascontiguousarray(Wo.reshape(CB, P, C).astype(bf16))
    # additive causal mask, [k, q] layout: block0 (q 0:197 vs k) at cols
    # 0:197 (only q<128 has a masked triangle), block1 (q'=q-128 vs k'=k-128)
    # at cols 197:266
    mask = np.zeros((P, SW), dtype=np.float32)
    kk = np.arange(P)[:, None]
    q0 = np.arange(P)[None, :]
    mask[:, 0:P] = np.where(q0 < kk, NEG, 0.0)
    q1 = np.arange(T1)[None, :]
    mask[:, T:SW] = np.where(q1 < kk, NEG, 0.0)
    ident = np.eye(P, dtype=np.float32)

    in_maps = []
    for c in range(NCORES):
        xs = x[c * NB:(c + 1) * NB]  # [NB, T, C]
        xTc = xs.reshape(NB, T, CB, P).transpose(2, 3, 0, 1).reshape(CB, P, NB * T)
        in_maps.append({
            "xT": np.ascontiguousarray(xTc.astype(bf16)),
            "wq": wq_h, "wk": wk_h, "wv": wv_h, "wo": wo_h,
            "mask": np.ascontiguousarray(mask.astype(bf16)),
            "ident": np.ascontiguousarray(ident.astype(bf16)),
        })
    return in_maps


def kernel(hidden_states, Wq, bq, Wk, bk, Wv, bv, Wo, bo, counter, ucb,
           **extra):
    hidden_states = np.asarray(hidden_states)
    Wq, bq = np.asarray(Wq), np.asarray(bq)
    Wk, bk = np.asarray(Wk), np.asarray(bk)
    Wv, bv = np.asarray(Wv), np.asarray(bv)
    Wo, bo = np.asarray(Wo), np.asarray(bo)

    if np.any(bq) or np.any(bk):
        # exact numpy fallback (not expected to trigger: spec fills zeros)
        return _numpy_reference(hidden_states, Wq, bq, Wk, bk, Wv, bv, Wo, bo)

    if "nc" not in _CACHE:
        _CACHE["nc"] = build_nc()
    nc = _CACHE["nc"]
    if "runner" not in _CACHE:
        _CACHE["runner"] = _make_runner(nc)
    run, out_names, out_avals = _CACHE["runner"]

    in_maps = _prep_core_inputs(hidden_states, Wq, Wk, Wv, Wo)
    out_arrs, _ = run(in_maps)
    full = np.asarray(out_arrs[out_names.index("out")])
    out = full  # [NCORES*NB, T, C] — concat over cores is exactly batch order

    # bv/bo enter the output linearly: out += bv @ Wo + bo (attention rows sum
    # to one, so the bv term is constant across positions).
    if np.any(bv) or np.any(bo):
        out = out + (bv.astype(np.float64) @ Wo.astype(np.float64)
                     + bo.astype(np.float64)).astype(np.float32)[None, None, :]
    return out.astype(np.float32)


def _make_runner(nc):
    """Cached jitted runner (mirrors bass2jax.run_bass_via_pjrt) that keeps
    inputs device-resident so repeated calls time pure device execution."""
    import jax
    import concourse.mybir as mybir
    from concourse import bass2jax
    from concourse.bass2jax import _bass_exec_p, install_neuronx_cc_hook
    from jax.sharding import Mesh, PartitionSpec
    from jax.experimental.shard_map import shard_map

    install_neuronx_cc_hook()
    n_cores = NCORES
    partition_name = (nc.partition_id_tensor.name
                      if nc.partition_id_tensor else None)
    in_names, out_names, out_avals = [], [], []
    for alloc in nc.m.functions[0].allocations:
        if not isinstance(alloc, mybir.MemoryLocationSet):
            continue
        name = alloc.memorylocations[0].name
        if alloc.kind == "ExternalInput":
            if name != partition_name:
                in_names.append(name)
        elif alloc.kind == "ExternalOutput":
            shape = tuple(alloc.tensor_shape)
            dtype = mybir.dt.np(alloc.dtype)
            out_names.append(name)
            out_avals.append(jax.core.ShapedArray(shape, dtype))
    n_params = len(in_names)
    all_names = in_names + out_names
    if partition_name is not None:
        all_names = all_names + [partition_name]

    def _body(*args):
        operands = list(args)
        if partition_name is not None:
            operands.append(bass2jax.partition_id_tensor())
        outs = _bass_exec_p.bind(
            *operands,
            out_avals=tuple(out_avals),
            in_names=tuple(all_names),
            out_names=tuple(out_names),
            lowering_input_output_aliases=(),
            sim_require_finite=False,
            sim_require_nnan=False,
            nc=nc,
        )
        return tuple(outs)

    devices = jax.devices()[:n_cores]
    mesh = Mesh(np.asarray(devices), ("core",))
    in_specs = (PartitionSpec("core"),) * (n_params + len(out_names))
    out_specs = (PartitionSpec("core"),) * len(out_names)
    sharded = jax.jit(
        shard_map(_body, mesh=mesh, in_specs=in_specs, out_specs=out_specs,
                  check_rep=False),
        keep_unused=True,
    )

    def make_repeat(repeat):
        n_outs = len(out_names)

        def _body_r(*args):
            params = list(args[:n_params])
            outbufs = list(args[n_params:])
            outs = None
            for _ in range(repeat):
                outs = _body(*params, *outbufs)
                outbufs = list(outs)
            return outs
        return jax.jit(
            shard_map(_body_r, mesh=mesh, in_specs=in_specs,
                      out_specs=out_specs, check_rep=False),
            keep_unused=True,
        )

    def run(in_maps, device_inputs=None):
        if device_inputs is None:
            concat_in = [
                np.concatenate([np.asarray(in_maps[c][nm]) for c in range(n_cores)],
                               axis=0)
                for nm in in_names
            ]
            concat_zeros = [
                np.zeros((n_cores * a.shape[0], *a.shape[1:]), a.dtype)
                for a in out_avals
            ]
            device_inputs = jax.device_put(
                concat_in + concat_zeros,
                [jax.sharding.NamedSharding(mesh, PartitionSpec("core"))]
                * (n_params + len(out_names)),
            )
        out_arrs = sharded(*device_inputs)
        jax.block_until_ready(out_arrs)
        return out_arrs, device_inputs

    run.make_repeat = make_repeat
    return run, out_names, out_avals


def _numpy_reference(hidden_states, Wq, bq, Wk, bk, Wv, bv, Wo, bo):
    x = hidden_states.astype(np.float64)
    q = (x @ Wq.astype(np.float64) + bq).reshape(B, T, H, D).transpose(0, 2, 1, 3)
    k = (x @ Wk.astype(np.float64) + bk).reshape(B, T, H, D).transpose(0, 2, 1, 3)
    v = (x @ Wv.astype(np.float64) + bv).reshape(B, T, H, D).transpose(0, 2, 1, 3)
    s = np.einsum("bhqd,bhkd->bhqk", q, k) / np.sqrt(D)
    causal = np.tril(np.ones((T, T), dtype=bool))
    s = np.where(causal, s, -np.inf)
    s = s - s.max(axis=-1, keepdims=True)
    p = np.exp(s)
    p = p / p.sum(axis=-1, keepdims=True)
    ctx = np.einsum("bhqk,bhkd->bhqd", p, v).transpose(0, 2, 1, 3).reshape(B, T, C)
    return (ctx @ Wo.astype(np.float64) + bo).astype(np.float32)
